# revision 71
# baseline (speedup 1.0000x reference)
"""Trainium2 Bass kernel v2 for the soft Bezier rasterizer.

dist^2(px, seg) = min(Wa, Wb, E + relu(Q)),  Q = |ab|^2 (t^2 - t);
E, Q, W (vertex dist^2), C (winding cross) are quadratics in px evaluated
by PE matmuls (split-bf16 weights, fp32 PSUM).  relu(Q) on ACT -> rq bf16;
an accumulating identity matmul adds rq onto the E columns in PSUM.
Per-(unit,shape) minima via bf16 tensor_tensor min fold-trees over
class-quantized group widths; units sorted by class so every fold level is
one big op.  Winding: g = [C>0] (tensor_scalar is_gt) + bf16 add-trees;
inside <=> sum != #dn.  Composite: premultiplied over-blend as a linear
recurrence via tensor_tensor_scan on a 9-slot [R, s0..s7] layout.
"""
import sys
import os
import numpy as np

for _p in ('/opt/trn_rl_repo',):
    if _p not in sys.path and os.path.isdir(_p):
        sys.path.insert(0, _p)

import ml_dtypes

BF16 = ml_dtypes.bfloat16

N = 8
S = 30
HW = 384
EPS = 1e-8
BIGD = 1e6
DTH = 0.04
NCORES = 8
RPC = HW // NCORES
CB = 3
NU = RPC * CB               # 144 units
NSL = NU * N                # 1152 slots
CHUNKCAP = 1536


def _bezier_to_polyline(cp, n_samples=S):
    t_global = np.linspace(0.0, 4.0 - 4.0 / n_samples, n_samples)
    seg = np.clip(np.floor(t_global).astype(np.int64), 0, 3)
    t = t_global - seg
    ti = 1.0 - t
    basis = np.stack([ti**3, 3*ti**2*t, 3*ti*t**2, t**3], axis=-1)
    idx = np.stack([seg*3, seg*3+1, seg*3+2, (seg*3+3) % 12], axis=-1)
    gathered = cp[:, idx, :]
    return np.einsum('sk,mskd->msd', basis, gathered)


def _split3(x):
    xh = x.astype(BF16).astype(np.float64)
    xm = (x - xh).astype(BF16).astype(np.float64)
    xl = (x - xh - xm).astype(BF16).astype(np.float64)
    return xh, xm, xl


_XTERM = [0, 0, 1, 0, 1, 2]
_WTERM = [0, 1, 0, 2, 1, 0]


def _qz(v, classes):
    for c in classes:
        if v <= c:
            return c
    return classes[-1]


def _runs(vals):
    """[(start, end, val)] maximal runs of equal values."""
    out = []
    s = 0
    for j in range(1, len(vals)+1):
        if j == len(vals) or vals[j] != vals[s]:
            out.append((s, j, vals[s]))
            s = j
    return out


def _precompute(P, c, alpha, alive, z):
    P64 = np.asarray(P, np.float64)
    sig_alive = 1.0 / (1.0 + np.exp(-np.asarray(alive, np.float64)))
    eff_alpha = np.where(sig_alive > 0.1, np.asarray(alpha, np.float64), 0.0)
    order = np.argsort(np.asarray(z, np.float64), kind='stable')
    P_s = P64[order]
    c_s = np.asarray(c, np.float64)[order]
    a_s = eff_alpha[order]

    poly = _bezier_to_polyline(P_s).astype(np.float32).astype(np.float64)
    a = poly
    b = np.roll(poly, -1, axis=1)
    ax, ay = a[..., 0].ravel(), a[..., 1].ravel()
    bx, by = b[..., 0].ravel(), b[..., 1].ravel()
    abx, aby = bx - ax, by - ay
    ab2 = abx**2 + aby**2 + EPS
    ylo, yhi = np.minimum(ay, by), np.maximum(ay, by)
    xlo, xhi = np.minimum(ax, bx), np.maximum(ax, bx)

    y = np.linspace(0.0, 1.0, HW)
    x = np.linspace(0.0, 1.0, HW)
    px0 = np.array([x[cb*128:(cb+1)*128].mean() for cb in range(CB)])

    elists, vlists, clists = {}, {}, {}
    ndn = np.zeros((HW, N))
    for r in range(HW):
        py = y[r]
        ys = (py > ylo - DTH) & (py < yhi + DTH)
        yv = np.abs(ay - py) <= DTH
        up = (ay <= py) & (py < by)
        dn = (ay > py) & (py >= by)
        cr = up | dn
        clists[r] = [np.nonzero(cr.reshape(N, S)[m])[0] for m in range(N)]
        ndn[r] = dn.reshape(N, S).sum(1)
        for cb in range(CB):
            x0b, x1b = x[cb*128], x[cb*128+127]
            xs = (xhi > x0b - DTH) & (xlo < x1b + DTH)
            xv = (ax > x0b - DTH) & (ax < x1b + DTH)
            es = (ys & xs).reshape(N, S)
            vs = (yv & xv).reshape(N, S)
            elists[(r, cb)] = [np.nonzero(es[m])[0] for m in range(N)]
            vlists[(r, cb)] = [np.nonzero(vs[m])[0] for m in range(N)]

    PE_CLS = (0, 2, 4, 8, 16)
    PV_CLS = (0, 2, 4, 8, 16)
    C_CLS = (2, 4, 8)
    units = []
    for i in range(RPC):
        rows = [i*NCORES + cc for cc in range(NCORES)]
        pc = max(len(clists[r][m]) for r in rows for m in range(N))
        cC = _qz(max(pc, 1), C_CLS)
        for cb in range(CB):
            pe = max(len(elists[(r, cb)][m]) for r in rows for m in range(N))
            pv = max(len(vlists[(r, cb)][m]) for r in rows for m in range(N))
            units.append(dict(i=i, cb=cb, pe=_qz(pe, PE_CLS),
                              pv=_qz(pv, PV_CLS), c=cC))
    order_u = sorted(range(NU), key=lambda j: (units[j]['pe'], units[j]['pv'],
                                               units[j]['c'], units[j]['cb'],
                                               units[j]['i']))
    su = [units[j] for j in order_u]

    # per-unit slab offsets (globally contiguous in sorted order)
    e_sl, w_sl, g_sl = [], [], []
    eo = wo = go = 0
    for u in su:
        e_sl.append(eo)
        w_sl.append(wo)
        g_sl.append(go)
        if u['pe'] > 2:
            eo += 8*(u['pe']//2)
        if u['pv'] > 2:
            wo += 8*(u['pv']//2)
        go += 8*u['c']
    ESLAB, WSLAB, GSLAB = max(eo, 1), max(wo, 1), max(go, 1)

    # chunks: consecutive sorted units, total psum cols <= CHUNKCAP
    chunks = []
    cur = None
    for j, u in enumerate(su):
        cols = 8*(2*u['pe'] + u['pv'] + u['c'])
        if cur is None or cur['cols'] + cols > CHUNKCAP:
            cur = dict(cols=0, u0=j, nu=0)
            chunks.append(cur)
        cur['cols'] += cols
        cur['nu'] += 1

    for ch in chunks:
        uu = su[ch['u0']:ch['u0']+ch['nu']]
        ch['nE'] = sum(8*u['pe'] for u in uu)
        ch['nW'] = sum(8*u['pv'] for u in uu)
        ch['nQ'] = ch['nE']
        ch['nC'] = sum(8*u['c'] for u in uu)
        ch['oW'], ch['oE'] = 0, ch['nW']
        ch['oQ'] = ch['oE'] + ch['nE']
        ch['oC'] = ch['oQ'] + ch['nQ']
        ch['tot'] = ch['oC'] + ch['nC']
        # mm pieces in data-arrival order: W (Vector's first consumer),
        # then E|Q (relu), then C (sign); split at cb changes and 512 grid
        pieces = []
        for zkey, cnt in (('oW', lambda u: 8*u['pv']),
                          ('oE', lambda u: 8*u['pe']),
                          ('oQ', lambda u: 8*u['pe']),
                          ('oC', lambda u: 8*u['c'])):
            zo = ch[zkey]
            off = zo
            c0 = zo
            for jj, u in enumerate(uu):
                if jj > 0 and uu[jj-1]['cb'] != u['cb']:
                    if off > c0:
                        pieces.append((c0, off, uu[jj-1]['cb']))
                    c0 = off
                off += cnt(u)
            if off > c0:
                pieces.append((c0, off, uu[-1]['cb']))
        split = []
        for (c0, c1, cbv) in pieces:
            p = c0
            while p < c1:
                nx = min(c1, (p//512+1)*512)
                split.append((p, nx, cbv))
                p = nx
        ch['mm'] = split
        acc = []
        p = ch['oE']
        while p < ch['oE']+ch['nE']:
            nx = min(ch['oE']+ch['nE'], (p//512+1)*512)
            acc.append((p, nx))
            p = nx
        ch['mmacc'] = acc
        # lvl0 fold pieces per class-run within chunk: (qty, psum off, G, w,
        # dstslab, dstoff).  dst 'E'/'W' slabs or 'mE'/'mW' when w==2.
        lvl0 = []
        for qty, zkey, key, mind in (('E', 'oE', 'pe', 'mE'),
                                     ('W', 'oW', 'pv', 'mW')):
            off = ch[zkey]
            vals = [u[key] for u in uu]
            for (s0, s1, v) in _runs(vals):
                G = (s1 - s0)*8
                if v > 0:
                    lvl0.append((qty, off, G, v, mind, (ch['u0']+s0)*8))
                off += G*v
        ch['lvl0'] = lvl0

    # C winding folds: per-chunk add-reduces over class runs (into sS fp32)
    g_off2 = 0
    for ch in chunks:
        uu = su[ch['u0']:ch['u0']+ch['nu']]
        cfold = []
        off = g_off2
        vals = [u['c'] for u in uu]
        for (s0, s1, v) in _runs(vals):
            G = (s1 - s0)*8
            cfold.append((off, G, v, (ch['u0']+s0)*8))
            off += G*v
        ch['cfold'] = cfold
        g_off2 = off

    # memset ranges for slots never written by folds (class 0)
    def zero_ranges(key):
        rngs = []
        for (s0, s1, v) in _runs([u[key] for u in su]):
            if v == 0:
                rngs.append((s0*8, (s1 - s0)*8))
        return rngs
    zeroE = zero_ranges('pe')
    zeroW = zero_ranges('pv')
    SCR = 1

    # ---- weights
    xf = np.stack([(x - np.repeat(px0, 128))**2,
                   x - np.repeat(px0, 128),
                   np.ones(HW)], 0)
    Xh, Xm, Xl = _split3(xf)
    X18 = np.zeros((18, CB, 128), BF16)
    for cb in range(CB):
        for t6 in range(6):
            X18[t6*3:(t6+1)*3, cb] = \
                (Xh, Xm, Xl)[_XTERM[t6]][:, cb*128:(cb+1)*128].astype(BF16)

    e_lin = aby*y[:, None] - abx*ax - aby*ay
    inv = 1.0 / ab2

    def col_coeffs(r, kind, g):
        py = y[r]
        n = len(g)
        Cq = np.zeros((3, n))
        if kind == 'E':
            e = e_lin[r][g]
            Cq[0] = 1.0 - abx[g]**2*inv[g]
            Cq[1] = -2*ax[g] - 2*abx[g]*e*inv[g]
            Cq[2] = ax[g]**2 + (py - ay[g])**2 - e**2*inv[g]
        elif kind == 'Q':
            e = e_lin[r][g]
            Cq[0] = abx[g]**2*inv[g]
            Cq[1] = 2*abx[g]*e*inv[g] - abx[g]
            Cq[2] = e**2*inv[g] - e
        elif kind == 'W':
            Cq[0] = 1.0
            Cq[1] = -2*ax[g]
            Cq[2] = ax[g]**2 + (py - ay[g])**2
        elif kind == 'C':
            Cq[1] = -aby[g]
            Cq[2] = abx[g]*(py - ay[g]) + ax[g]*aby[g]
        return Cq

    choff = []
    o = 0
    eb = 0
    for ch in chunks:
        choff.append(o)
        o += ch['tot']
        ch['e_base'] = eb
        eb += ch['nE']
    TOTW = o
    ETOT = max(eb, 1)
    Wcore = np.zeros((NCORES, 18, TOTW), BF16)
    ck2 = np.zeros((NCORES, NSL), np.float32)

    def bake(Wd, col0, r, cb, kind, g, nsplit):
        n = len(g)
        if n == 0:
            return
        Cq = col_coeffs(r, kind, g)
        p0 = px0[cb]
        A, B_, C0 = Cq
        Wq = np.stack([A, 2*A*p0 + B_, A*p0*p0 + B_*p0 + C0], 0)
        parts = _split3(Wq)
        for t6 in range(nsplit):
            Wd[t6*3:(t6+1)*3, col0:col0+n] = parts[_WTERM[t6]].astype(BF16)

    for cc in range(NCORES):
        Wd = Wcore[cc]
        for ci, ch in enumerate(chunks):
            base = choff[ci]
            offs = {k: base + ch[k] for k in ('oE', 'oW', 'oQ', 'oC')}
            for jj in range(ch['nu']):
                u = su[ch['u0']+jj]
                r = u['i']*NCORES + cc
                cb = u['cb']
                pe, pv, cc_ = u['pe'], u['pv'], u['c']
                for m in range(N):
                    el = elists[(r, cb)][m]
                    vl = vlists[(r, cb)][m]
                    cl = clists[r][m]
                    sl = m*S
                    c0 = offs['oE'] + m*pe
                    bake(Wd, c0, r, cb, 'E', sl+el, 6)
                    Wd[2, c0+len(el):c0+pe] = BF16(BIGD)
                    c0 = offs['oW'] + m*pv
                    bake(Wd, c0, r, cb, 'W', sl+vl, 6)
                    Wd[2, c0+len(vl):c0+pv] = BF16(BIGD)
                    c0 = offs['oQ'] + m*pe
                    bake(Wd, c0, r, cb, 'Q', sl+el, 3)
                    c0 = offs['oC'] + m*cc_
                    bake(Wd, c0, r, cb, 'C', sl+cl, 3)
                    ck2[cc, (ch['u0']+jj)*8 + m] = 2*ndn[r][m] - len(cl)
                offs['oE'] += 8*pe
                offs['oW'] += 8*pv
                offs['oQ'] += 8*pe
                offs['oC'] += 8*cc_

    al8 = (-a_s).astype(np.float32)                 # (8,)
    ca8 = (c_s.T * a_s[None, :]).astype(np.float32)  # (3, 8)

    return dict(su=su, order_u=order_u, chunks=chunks, choff=choff,
                Wcore=Wcore, X18=X18, ck2=ck2,
                al8=al8, ca8=ca8, zeroE=zeroE, zeroW=zeroW,
                ESLAB=ESLAB, WSLAB=WSLAB, GSLAB=GSLAB, SCR=SCR,
                TOTW=TOTW, ETOT=ETOT, a_s=a_s, c_s=c_s)


# ----------------------------------------------------------- numpy emulator
def _bf(x):
    return x.astype(BF16).astype(np.float32)


def _emulate(pre, core):
    X = pre['X18'].astype(np.float32)
    Wd = pre['Wcore'][core].astype(np.float32)
    mindE = np.full((128, NSL), BIGD, np.float32)
    mindW = np.full((128, NSL), BIGD, np.float32)
    sEa = np.zeros((128, pre['ETOT']), np.float32)
    sS = np.zeros((128, NSL), np.float32)
    for ci, ch in enumerate(pre['chunks']):
        base = pre['choff'][ci]
        psum = np.zeros((128, ch['tot']), np.float32)
        for (c0, c1, cbv) in ch['mm']:
            psum[:, c0:c1] = X[:, cbv, :].T @ Wd[:, base+c0:base+c1]
        rq = _bf(np.maximum(psum[:, ch['oQ']:ch['oQ']+ch['nQ']], 0.0))
        psum[:, ch['oE']:ch['oE']+ch['nE']] += rq
        sg = _bf(np.sign(psum[:, ch['oC']:ch['oC']+ch['nC']]))
        for (qty, off, G, w, dst, doff) in ch['lvl0']:
            A = psum[:, off:off+G*w].reshape(128, G, w)
            out = _bf(A.min(axis=2))
            (mindE if dst == 'mE' else mindW)[:, doff:doff+G] = out
        goff0 = ch['cfold'][0][0] if ch['cfold'] else 0
        for (goff, G, w, doff) in ch['cfold']:
            A = sg[:, goff-goff0:goff-goff0+G*w].reshape(128, G, w)
            sS[:, doff:doff+G] = A.sum(axis=2)
    mind2 = _bf(np.minimum(mindE, mindW))
    m0 = np.maximum(mind2, 0.0)
    sd = _bf(np.sqrt(m0 + EPS))
    eq = (sS == pre['ck2'][core][None, :]).astype(np.float32)
    sgn = eq*2.0 - 1.0
    sdf = _bf(sgn*sd)
    cov = 1.0/(1.0 + np.exp(np.clip(100.0*sdf, -80, 80)))
    cov = _bf(cov).reshape(128, NU, N)
    uu = _bf(_bf(cov*_bf(pre['al8'])[None, None, :]) + 1.0)
    uu[:, :, 0] = 0.0
    out = np.zeros((128, 3, NU), np.float32)
    for chn in range(3):
        dd = _bf(cov*_bf(pre['ca8'][chn])[None, None, :])
        st = np.zeros((128, NU), np.float32)
        for sl in range(N):
            st = _bf(uu[:, :, sl]*st + dd[:, :, sl])
        out[:, chn] = np.clip(st, 0.0, 1.0)
    return out


def _assemble(pre, outs):
    img = np.empty((HW, HW, 3), np.float32)
    for cc in range(NCORES):
        o = outs[cc]
        for j in range(NU):
            u = pre['su'][j]
            r = u['i']*NCORES + cc
            cb = u['cb']
            img[r, cb*128:(cb+1)*128, :] = o[:, :, j]
    return img


# ------------------------------------------------------------- bass program
def _build_program(pre):
    import concourse.bass as bass
    import concourse.bacc as bacc
    import concourse.mybir as mybir
    from concourse import tile

    dt = mybir.dt.float32
    bt = mybir.dt.bfloat16
    AF = mybir.ActivationFunctionType
    ALU = mybir.AluOpType
    AX = mybir.AxisListType

    chunks, choff = pre['chunks'], pre['choff']

    nc = bacc.Bacc()
    w_d = nc.declare_dram_parameter("w", [18, pre['TOTW']], bt, isOutput=False)
    xf_d = nc.declare_dram_parameter("xfeat", [18, CB, 128], bt,
                                     isOutput=False)
    ck2_d = nc.declare_dram_parameter("ck2", [128, NSL], dt, isOutput=False)
    al8_d = nc.declare_dram_parameter("al8", [128, N], bt, isOutput=False)
    ca8_d = nc.declare_dram_parameter("ca8", [128, 3, N], bt, isOutput=False)
    idf_d = nc.declare_dram_parameter("identf", [128, 128], dt,
                                      isOutput=False)
    cst_d = nc.declare_dram_parameter("consts", [128, 8], dt, isOutput=False)
    out_d = nc.declare_dram_parameter("out", [3, NU, 128], dt, isOutput=True)

    with tile.TileContext(nc) as tc:
        with (
            tc.tile_pool(name="const", bufs=1) as cpool,
            tc.tile_pool(name="wpool", bufs=4) as wpool,
            tc.tile_pool(name="rqp", bufs=4) as rqp,
            tc.tile_pool(name="slabs", bufs=1) as slabs,
            tc.tile_pool(name="work", bufs=2) as work,
            tc.tile_pool(name="psc", bufs=2, space=bass.MemorySpace.PSUM) as psc,
            tc.tile_pool(name="pst", bufs=1, space=bass.MemorySpace.PSUM) as pst,
        ):
            # critical-path consts first; fat consts are DMA'd mid-loop
            xfeat = cpool.tile([18, CB, 128], bt)
            nc.sync.dma_start(xfeat[:], xf_d[:])
            cst = cpool.tile([128, 8], dt)
            c_eps = cst[:, 0:1]
            identf = cpool.tile([128, 128], dt)
            ck2t = cpool.tile([128, NSL], dt)
            al8 = cpool.tile([128, N], bt)
            ca8 = cpool.tile([128, 3, N], bt)

            sG = slabs.tile([128, pre['GSLAB']], bt)
            sEa = slabs.tile([128, pre['ETOT']], bt)
            mE = slabs.tile([128, NSL], bt)
            mW = slabs.tile([128, NSL], bt)
            sS = slabs.tile([128, NSL], dt)
            for (off, ln) in pre['zeroE']:
                nc.vector.memset(mE[:, off:off+ln], BIGD)
            for (off, ln) in pre['zeroW']:
                nc.vector.memset(mW[:, off:off+ln], BIGD)

            smap = {'mE': mE, 'mW': mW}

            def view3(t, off, G, w):
                return t[:, off:off+G*w].rearrange("p (g w) -> p g w", w=w)

            # pre-sigmoid elementwise chain (no ACT tables) emitted per slot
            # range; the bulk runs mid-loop in Vector idle slack
            mind2 = slabs.tile([128, NSL], bt)
            m0 = slabs.tile([128, NSL], bt)
            eq = slabs.tile([128, NSL], bt)
            sgn = slabs.tile([128, NSL], bt)

            def emit_pre(s0, s1):
                nc.vector.tensor_tensor(mind2[:, s0:s1], mE[:, s0:s1],
                                        mW[:, s0:s1], ALU.min)
                nc.vector.tensor_scalar_max(m0[:, s0:s1], mind2[:, s0:s1],
                                            0.0)
                nc.vector.tensor_tensor(eq[:, s0:s1], sS[:, s0:s1],
                                        ck2t[:, s0:s1], ALU.is_equal)
                nc.vector.tensor_scalar(sgn[:, s0:s1], eq[:, s0:s1],
                                        2.0, -1.0, ALU.mult, ALU.add)

            PREK = min(16, len(chunks) - 2)
            chK = chunks[PREK]
            PRESL = (chK['u0'] + chK['nu'])*8

            # ---------------- main loop
            g_off = 0
            for ci, ch in enumerate(chunks):
                base = choff[ci]
                wt = wpool.tile([18, ch['tot']], bt, tag="w")
                nc.sync.dma_start(wt[:], w_d[:, base:base+ch['tot']])
                ps = psc.tile([128, ch['tot']], dt, tag="ps")
                for (c0, c1, cbv) in ch['mm']:
                    nc.tensor.matmul(ps[:, c0:c1], xfeat[:, cbv, :],
                                     wt[:, c0:c1], start=True, stop=True)
                if ch['nQ']:
                    # E and Q zones are adjacent: one relu covers both
                    # (relu on E only clips negative rounding noise)
                    ebrq = rqp.tile([128, ch['nE']+ch['nQ']], bt, tag="ebrq")
                    nc.scalar.activation(ebrq[:],
                                         ps[:, ch['oE']:ch['oE']+ch['nE'] +
                                            ch['nQ']],
                                         AF.Relu)
                if ch['nC']:
                    nc.scalar.sign(
                        sG[:, g_off:g_off+ch['nC']],
                        ps[:, ch['oC']:ch['oC']+ch['nC']])
                # V queue: psum-only consumers (W reduces, C folds) first so
                # they overlap the Scalar relu/copy chain; E path after
                for (qty, off, G, w, dst, doff) in ch['lvl0']:
                    if qty == 'W':
                        nc.vector.tensor_reduce(
                            smap[dst][:, doff:doff+G], view3(ps, off, G, w),
                            AX.X, ALU.min)
                if ch['nQ']:
                    nc.vector.tensor_tensor(
                        sEa[:, ch['e_base']:ch['e_base']+ch['nE']],
                        ebrq[:, 0:ch['nE']],
                        ebrq[:, ch['nE']:ch['nE']+ch['nQ']], ALU.add)
                for (qty, off, G, w, dst, doff) in ch['lvl0']:
                    if qty == 'E':
                        so = ch['e_base'] + (off - ch['oE'])
                        nc.vector.tensor_reduce(
                            smap[dst][:, doff:doff+G], view3(sEa, so, G, w),
                            AX.X, ALU.min)
                for (goff, G, w, doff) in ch['cfold']:
                    nc.vector.tensor_reduce(
                        sS[:, doff:doff+G], view3(sG, goff, G, w),
                        AX.X, ALU.add)
                g_off += ch['nC']
                if ci == 3:
                    nc.sync.dma_start(cst[:], cst_d[:])
                    nc.sync.dma_start(ck2t[:], ck2_d[:])
                    nc.sync.dma_start(al8[:], al8_d[:])
                    nc.sync.dma_start(ca8[:], ca8_d[:])
                    nc.sync.dma_start(identf[:], idf_d[:])
                if ci == PREK:
                    emit_pre(0, PRESL)

            # ---------------- end phase, pipelined in two slot batches:
            # batch 1's sqrt starts right after the last relu while batch 2's
            # pre-chain still runs; each stage of one batch overlaps the
            # other batch's neighbor stage
            emit_pre(PRESL, NSL)
            sd = slabs.tile([128, NSL], bt)
            sdf = slabs.tile([128, NSL], bt)
            cov = slabs.tile([128, NSL], bt)
            uu = slabs.tile([128, NU, N], bt)
            och = slabs.tile([128, 3, NU], dt)
            d1_0 = slabs.tile([128, NU, N], bt)
            d1_1 = slabs.tile([128, NU, N], bt)
            d1_2 = slabs.tile([128, NU, N], bt)
            sc_0 = slabs.tile([128, NU, N], bt)
            sc_1 = slabs.tile([128, NU, N], bt)
            sc_2 = slabs.tile([128, NU, N], bt)
            d1s = [d1_0, d1_1, d1_2]
            scs = [sc_0, sc_1, sc_2]
            cov3 = cov[:].rearrange("p (u n) -> p u n", n=N)
            al8b = al8[:].rearrange("p (u n) -> p u n", u=1)\
                .broadcast_to([128, NU, N])
            batches = [(0, PRESL), (PRESL, NSL)]
            for (s0, s1) in batches:
                nc.scalar.activation(sd[:, s0:s1], m0[:, s0:s1], AF.Sqrt,
                                     bias=c_eps)
            for (s0, s1) in batches:
                nc.vector.tensor_tensor(sdf[:, s0:s1], sgn[:, s0:s1],
                                        sd[:, s0:s1], ALU.mult)
            for (s0, s1) in batches:
                nc.scalar.activation(cov[:, s0:s1], sdf[:, s0:s1],
                                     AF.Sigmoid, scale=-100.0)
                u0, u1 = s0//N, s1//N
                nc.vector.tensor_tensor(uu[:, u0:u1], cov3[:, u0:u1],
                                        al8b[:, u0:u1], ALU.mult)
                nc.vector.tensor_scalar(uu[:, u0:u1], uu[:, u0:u1],
                                        1.0, None, ALU.add)
                nc.vector.memset(uu[:, u0:u1, 0:1], 0.0)
                for chn in range(3):
                    ca8b = ca8[:, chn, :]\
                        .rearrange("p (u n) -> p u n", u=1)\
                        .broadcast_to([128, NU, N])
                    # on GpSimd: off Vector's critical path; overlaps the
                    # uu chain and the scans (SBUF-only, so Pool is legal)
                    nc.gpsimd.tensor_tensor(d1s[chn][:, u0:u1],
                                            cov3[:, u0:u1],
                                            ca8b[:, u0:u1], ALU.mult)
            # scan -> clip -> transpose -> copy per channel so the output
            # path of channel k overlaps channel k+1's scan
            obig = work.tile([128, 3, 128], dt, tag="obig")
            osml = work.tile([16, 3, 128], dt, tag="osml")
            uflat = uu[:].rearrange("p u n -> p (u n)")
            for chn in range(3):
                nc.vector.tensor_tensor_scan(
                    scs[chn][:].rearrange("p u n -> p (u n)"), uflat,
                    d1s[chn][:].rearrange("p u n -> p (u n)"), 0.0,
                    ALU.mult, ALU.add)
                nc.vector.tensor_scalar(och[:, chn, :], scs[chn][:, :, N-1],
                                        0.0, 1.0, ALU.max, ALU.min)
                t1 = pst.tile([128, 128], dt, tag="t1")
                nc.tensor.transpose(t1[:], och[:, chn, 0:128], identf[:])
                nc.scalar.copy(obig[:, chn, :], t1[:])
                t2 = pst.tile([16, 128], dt, tag="t2")
                nc.tensor.transpose(t2[:], och[:, chn, 128:NU], identf[:])
                nc.scalar.copy(osml[:, chn, :], t2[:])
            nc.sync.dma_start(
                out_d[:, 0:128, :].rearrange("c u p -> u c p"), obig[:])
            nc.sync.dma_start(
                out_d[:, 128:NU, :].rearrange("c u p -> u c p"), osml[:])

    nc.compile()
    return nc


# ---------------------------------------------------------------- fallback
def _numpy_reference(P, c, alpha, alive, z, csg, width, height):
    P = np.asarray(P, np.float32)
    sig = 1.0 / (1.0 + np.exp(-np.asarray(alive, np.float64)))
    eff_alpha = np.where(sig > 0.1, np.asarray(alpha, np.float64), 0.0)
    order = np.argsort(np.asarray(z, np.float64), kind='stable')
    P_s, c_s = P[order], np.asarray(c, np.float64)[order]
    a_s, csg_s = eff_alpha[order], np.asarray(csg, bool)[order]
    poly = _bezier_to_polyline(P_s.astype(np.float64))
    a = poly
    b = np.roll(poly, -1, axis=1)
    y = np.linspace(0, 1, height)
    x = np.linspace(0, 1, width)
    gx, gy = np.meshgrid(x, y)
    p = np.stack([gx, gy], -1)[None, None]
    av = a[:, :, None, None, :]
    bv = b[:, :, None, None, :]
    ab = bv - av
    ap = p - av
    t = np.clip((ap*ab).sum(-1) / ((ab*ab).sum(-1) + EPS), 0, 1)
    diff = p - (av + t[..., None]*ab)
    dist = np.sqrt((diff*diff).sum(-1).min(1) + EPS)
    ay_, by_, py_ = av[..., 1], bv[..., 1], p[..., 1]
    ax_, bx_, px_ = av[..., 0], bv[..., 0], p[..., 0]
    up = (ay_ <= py_) & (py_ < by_)
    dn = (ay_ > py_) & (py_ >= by_)
    left = (bx_-ax_)*(py_-ay_) - (px_-ax_)*(by_-ay_) > 0
    w = np.where(up & left, 1.0, 0.0) + np.where(dn & ~left, -1.0, 0.0)
    wn = w.sum(1)
    sdf = np.where(wn != 0, -dist, dist)
    cov = 1.0/(1.0 + np.exp(np.clip(sdf/0.01, -80, 80)))
    la_all = cov * a_s[:, None, None]
    rgb = np.zeros((height, width, 3))
    ca = np.zeros((height, width, 1))
    for s in range(len(a_s)):
        la = la_all[s][..., None]
        if csg_s[s]:
            ca2 = ca*(1-la)
            rgb = rgb * (ca2 > 0)
            ca = ca2
        else:
            out_a = la + ca*(1-la)
            safe = np.where(out_a > 0, out_a, 1.0)
            rgb = np.where(out_a > 0, (c_s[s]*la + rgb*ca*(1-la))/safe, 0.0)
            ca = out_a
    return np.clip(rgb*ca, 0, 1).astype(np.float32)


# ------------------------------------------------------------------ driver
LAST_RESULT = None


def kernel(P, c, alpha, alive, z, csg, width, height):
    global LAST_RESULT
    width = int(width)
    height = int(height)
    if width != HW or height != HW or np.asarray(csg).any():
        return _numpy_reference(P, c, alpha, alive, z, csg, width, height)

    pre = _precompute(P, c, alpha, alive, z)
    nc = _build_program(pre)

    from concourse.bass_utils import run_bass_kernel_spmd

    identf = np.eye(128, dtype=np.float32)
    cvals = [EPS] + [0.0]*7
    consts = np.broadcast_to(np.asarray(cvals, np.float32)[None, :],
                             (128, 8)).copy()
    al8 = np.broadcast_to(pre['al8'][None, :].astype(BF16), (128, N)).copy()
    ca8 = np.ascontiguousarray(np.broadcast_to(
        pre['ca8'][None, :, :].astype(BF16), (128, 3, N)))
    in_maps = []
    for cc in range(NCORES):
        ck2 = np.broadcast_to(pre['ck2'][cc][None, :].astype(np.float32),
                              (128, NSL)).copy()
        in_maps.append(dict(w=np.ascontiguousarray(pre['Wcore'][cc]),
                            xfeat=pre['X18'], ck2=ck2, al8=al8, ca8=ca8,
                            identf=identf, consts=consts))

    trace = bool(int(os.environ.get('DIFFRAST_TRACE', '0')))
    res = run_bass_kernel_spmd(nc, in_maps, core_ids=list(range(NCORES)),
                               trace=trace)
    LAST_RESULT = res

    outs = []
    for cc in range(NCORES):
        o = res.results[cc]['out']          # (3, NU, 128)
        outs.append(np.ascontiguousarray(o.transpose(2, 0, 1)))
    return _assemble(pre, outs)


if __name__ == '__main__':
    d = np.load(os.path.join(os.path.dirname(__file__), '_ref_cache.npz'))
    pre = _precompute(d['P'], d['c'], d['alpha'], d['alive'], d['z'])
    outs = [_emulate(pre, cc) for cc in range(NCORES)]
    img = _assemble(pre, outs)
    exp = d['expected']
    rel = np.linalg.norm(img - exp)/np.linalg.norm(exp)
    print('emulator rel err:', rel)
    print('chunks:', len(pre['chunks']),
          'mm:', sum(len(c['mm']) for c in pre['chunks']),
          'lvl0:', sum(len(c['lvl0']) for c in pre['chunks']),
          'cfold:', sum(len(c['cfold']) for c in pre['chunks']))



# revision 72
# speedup vs baseline: 1.0332x; 1.0332x over previous
"""Trainium2 Bass kernel v2 for the soft Bezier rasterizer.

dist^2(px, seg) = min(Wa, Wb, E + relu(Q)),  Q = |ab|^2 (t^2 - t);
E, Q, W (vertex dist^2), C (winding cross) are quadratics in px evaluated
by PE matmuls (split-bf16 weights, fp32 PSUM).  relu(Q) on ACT -> rq bf16;
an accumulating identity matmul adds rq onto the E columns in PSUM.
Per-(unit,shape) minima via bf16 tensor_tensor min fold-trees over
class-quantized group widths; units sorted by class so every fold level is
one big op.  Winding: g = [C>0] (tensor_scalar is_gt) + bf16 add-trees;
inside <=> sum != #dn.  Composite: premultiplied over-blend as a linear
recurrence via tensor_tensor_scan on a 9-slot [R, s0..s7] layout.
"""
import sys
import os
import numpy as np

for _p in ('/opt/trn_rl_repo',):
    if _p not in sys.path and os.path.isdir(_p):
        sys.path.insert(0, _p)

import ml_dtypes

BF16 = ml_dtypes.bfloat16

N = 8
S = 30
HW = 384
EPS = 1e-8
BIGD = 1e6
DTH = 0.04
NCORES = 8
RPC = HW // NCORES
CB = 3
NU = RPC * CB               # 144 units
NSL = NU * N                # 1152 slots
CHUNKCAP = 1536


def _bezier_to_polyline(cp, n_samples=S):
    t_global = np.linspace(0.0, 4.0 - 4.0 / n_samples, n_samples)
    seg = np.clip(np.floor(t_global).astype(np.int64), 0, 3)
    t = t_global - seg
    ti = 1.0 - t
    basis = np.stack([ti**3, 3*ti**2*t, 3*ti*t**2, t**3], axis=-1)
    idx = np.stack([seg*3, seg*3+1, seg*3+2, (seg*3+3) % 12], axis=-1)
    gathered = cp[:, idx, :]
    return np.einsum('sk,mskd->msd', basis, gathered)


def _split3(x):
    xh = x.astype(BF16).astype(np.float64)
    xm = (x - xh).astype(BF16).astype(np.float64)
    xl = (x - xh - xm).astype(BF16).astype(np.float64)
    return xh, xm, xl


_XTERM = [0, 0, 1, 0, 1, 2]
_WTERM = [0, 1, 0, 2, 1, 0]


def _qz(v, classes):
    for c in classes:
        if v <= c:
            return c
    return classes[-1]


def _runs(vals):
    """[(start, end, val)] maximal runs of equal values."""
    out = []
    s = 0
    for j in range(1, len(vals)+1):
        if j == len(vals) or vals[j] != vals[s]:
            out.append((s, j, vals[s]))
            s = j
    return out


def _precompute(P, c, alpha, alive, z):
    P64 = np.asarray(P, np.float64)
    sig_alive = 1.0 / (1.0 + np.exp(-np.asarray(alive, np.float64)))
    eff_alpha = np.where(sig_alive > 0.1, np.asarray(alpha, np.float64), 0.0)
    order = np.argsort(np.asarray(z, np.float64), kind='stable')
    P_s = P64[order]
    c_s = np.asarray(c, np.float64)[order]
    a_s = eff_alpha[order]

    poly = _bezier_to_polyline(P_s).astype(np.float32).astype(np.float64)
    a = poly
    b = np.roll(poly, -1, axis=1)
    ax, ay = a[..., 0].ravel(), a[..., 1].ravel()
    bx, by = b[..., 0].ravel(), b[..., 1].ravel()
    abx, aby = bx - ax, by - ay
    ab2 = abx**2 + aby**2 + EPS
    ylo, yhi = np.minimum(ay, by), np.maximum(ay, by)
    xlo, xhi = np.minimum(ax, bx), np.maximum(ax, bx)

    y = np.linspace(0.0, 1.0, HW)
    x = np.linspace(0.0, 1.0, HW)
    px0 = np.array([x[cb*128:(cb+1)*128].mean() for cb in range(CB)])

    elists, vlists, clists = {}, {}, {}
    ndn = np.zeros((HW, N))
    for r in range(HW):
        py = y[r]
        ys = (py > ylo - DTH) & (py < yhi + DTH)
        yv = np.abs(ay - py) <= DTH
        up = (ay <= py) & (py < by)
        dn = (ay > py) & (py >= by)
        cr = up | dn
        clists[r] = [np.nonzero(cr.reshape(N, S)[m])[0] for m in range(N)]
        ndn[r] = dn.reshape(N, S).sum(1)
        for cb in range(CB):
            x0b, x1b = x[cb*128], x[cb*128+127]
            xs = (xhi > x0b - DTH) & (xlo < x1b + DTH)
            xv = (ax > x0b - DTH) & (ax < x1b + DTH)
            es = (ys & xs).reshape(N, S)
            vs = (yv & xv).reshape(N, S)
            elists[(r, cb)] = [np.nonzero(es[m])[0] for m in range(N)]
            vlists[(r, cb)] = [np.nonzero(vs[m])[0] for m in range(N)]

    PE_CLS = (0, 2, 4, 8, 16)
    PV_CLS = (0, 2, 4, 8, 16)
    C_CLS = (2, 4, 8)
    units = []
    for i in range(RPC):
        rows = [i*NCORES + cc for cc in range(NCORES)]
        pc = max(len(clists[r][m]) for r in rows for m in range(N))
        cC = _qz(max(pc, 1), C_CLS)
        for cb in range(CB):
            pe = max(len(elists[(r, cb)][m]) for r in rows for m in range(N))
            pv = max(len(vlists[(r, cb)][m]) for r in rows for m in range(N))
            units.append(dict(i=i, cb=cb, pe=_qz(pe, PE_CLS),
                              pv=_qz(pv, PV_CLS), c=cC))
    order_u = sorted(range(NU), key=lambda j: (units[j]['pe'], units[j]['pv'],
                                               units[j]['c'], units[j]['cb'],
                                               units[j]['i']))
    su = [units[j] for j in order_u]

    # per-unit slab offsets (globally contiguous in sorted order)
    e_sl, w_sl, g_sl = [], [], []
    eo = wo = go = 0
    for u in su:
        e_sl.append(eo)
        w_sl.append(wo)
        g_sl.append(go)
        if u['pe'] > 2:
            eo += 8*(u['pe']//2)
        if u['pv'] > 2:
            wo += 8*(u['pv']//2)
        go += 8*u['c']
    ESLAB, WSLAB, GSLAB = max(eo, 1), max(wo, 1), max(go, 1)

    # chunks: consecutive sorted units, total psum cols <= CHUNKCAP
    chunks = []
    cur = None
    for j, u in enumerate(su):
        cols = 8*(2*u['pe'] + u['pv'] + u['c'])
        if cur is None or cur['cols'] + cols > CHUNKCAP:
            cur = dict(cols=0, u0=j, nu=0)
            chunks.append(cur)
        cur['cols'] += cols
        cur['nu'] += 1

    for ch in chunks:
        uu = su[ch['u0']:ch['u0']+ch['nu']]
        ch['nE'] = sum(8*u['pe'] for u in uu)
        ch['nW'] = sum(8*u['pv'] for u in uu)
        ch['nQ'] = ch['nE']
        ch['nC'] = sum(8*u['c'] for u in uu)
        ch['oW'], ch['oE'] = 0, ch['nW']
        ch['oQ'] = ch['oE'] + ch['nE']
        ch['oC'] = ch['oQ'] + ch['nQ']
        ch['tot'] = ch['oC'] + ch['nC']
        # mm pieces in data-arrival order: W (Vector's first consumer),
        # then E|Q (relu), then C (sign); split at cb changes and 512 grid
        pieces = []
        for zkey, cnt in (('oW', lambda u: 8*u['pv']),
                          ('oE', lambda u: 8*u['pe']),
                          ('oQ', lambda u: 8*u['pe']),
                          ('oC', lambda u: 8*u['c'])):
            zo = ch[zkey]
            off = zo
            c0 = zo
            for jj, u in enumerate(uu):
                if jj > 0 and uu[jj-1]['cb'] != u['cb']:
                    if off > c0:
                        pieces.append((c0, off, uu[jj-1]['cb']))
                    c0 = off
                off += cnt(u)
            if off > c0:
                pieces.append((c0, off, uu[-1]['cb']))
        split = []
        for (c0, c1, cbv) in pieces:
            p = c0
            while p < c1:
                nx = min(c1, (p//512+1)*512)
                split.append((p, nx, cbv))
                p = nx
        ch['mm'] = split
        acc = []
        p = ch['oE']
        while p < ch['oE']+ch['nE']:
            nx = min(ch['oE']+ch['nE'], (p//512+1)*512)
            acc.append((p, nx))
            p = nx
        ch['mmacc'] = acc
        # lvl0 fold pieces per class-run within chunk: (qty, psum off, G, w,
        # dstslab, dstoff).  dst 'E'/'W' slabs or 'mE'/'mW' when w==2.
        lvl0 = []
        for qty, zkey, key, mind in (('E', 'oE', 'pe', 'mE'),
                                     ('W', 'oW', 'pv', 'mW')):
            off = ch[zkey]
            vals = [u[key] for u in uu]
            for (s0, s1, v) in _runs(vals):
                G = (s1 - s0)*8
                if v > 0:
                    lvl0.append((qty, off, G, v, mind, (ch['u0']+s0)*8))
                off += G*v
        ch['lvl0'] = lvl0

    # C winding folds: per-chunk add-reduces over class runs (into sS fp32)
    g_off2 = 0
    for ch in chunks:
        uu = su[ch['u0']:ch['u0']+ch['nu']]
        cfold = []
        off = g_off2
        vals = [u['c'] for u in uu]
        for (s0, s1, v) in _runs(vals):
            G = (s1 - s0)*8
            cfold.append((off, G, v, (ch['u0']+s0)*8))
            off += G*v
        ch['cfold'] = cfold
        g_off2 = off

    # memset ranges for slots never written by folds (class 0)
    def zero_ranges(key):
        rngs = []
        for (s0, s1, v) in _runs([u[key] for u in su]):
            if v == 0:
                rngs.append((s0*8, (s1 - s0)*8))
        return rngs
    zeroE = zero_ranges('pe')
    zeroW = zero_ranges('pv')
    SCR = 1

    # ---- weights
    xf = np.stack([(x - np.repeat(px0, 128))**2,
                   x - np.repeat(px0, 128),
                   np.ones(HW)], 0)
    Xh, Xm, Xl = _split3(xf)
    X18 = np.zeros((18, CB, 128), BF16)
    for cb in range(CB):
        for t6 in range(6):
            X18[t6*3:(t6+1)*3, cb] = \
                (Xh, Xm, Xl)[_XTERM[t6]][:, cb*128:(cb+1)*128].astype(BF16)

    e_lin = aby*y[:, None] - abx*ax - aby*ay
    inv = 1.0 / ab2

    def col_coeffs(r, kind, g):
        py = y[r]
        n = len(g)
        Cq = np.zeros((3, n))
        if kind == 'E':
            e = e_lin[r][g]
            Cq[0] = 1.0 - abx[g]**2*inv[g]
            Cq[1] = -2*ax[g] - 2*abx[g]*e*inv[g]
            Cq[2] = ax[g]**2 + (py - ay[g])**2 - e**2*inv[g]
        elif kind == 'Q':
            e = e_lin[r][g]
            Cq[0] = abx[g]**2*inv[g]
            Cq[1] = 2*abx[g]*e*inv[g] - abx[g]
            Cq[2] = e**2*inv[g] - e
        elif kind == 'W':
            Cq[0] = 1.0
            Cq[1] = -2*ax[g]
            Cq[2] = ax[g]**2 + (py - ay[g])**2
        elif kind == 'C':
            Cq[1] = -aby[g]
            Cq[2] = abx[g]*(py - ay[g]) + ax[g]*aby[g]
        return Cq

    choff = []
    o = 0
    eb = 0
    for ch in chunks:
        choff.append(o)
        o += ch['tot']
        ch['e_base'] = eb
        eb += ch['nE']
    TOTW = o
    ETOT = max(eb, 1)
    Wcore = np.zeros((NCORES, 18, TOTW), BF16)
    ck2 = np.zeros((NCORES, NSL), np.float32)

    def bake(Wd, col0, r, cb, kind, g, nsplit):
        n = len(g)
        if n == 0:
            return
        Cq = col_coeffs(r, kind, g)
        p0 = px0[cb]
        A, B_, C0 = Cq
        Wq = np.stack([A, 2*A*p0 + B_, A*p0*p0 + B_*p0 + C0], 0)
        parts = _split3(Wq)
        for t6 in range(nsplit):
            Wd[t6*3:(t6+1)*3, col0:col0+n] = parts[_WTERM[t6]].astype(BF16)

    for cc in range(NCORES):
        Wd = Wcore[cc]
        for ci, ch in enumerate(chunks):
            base = choff[ci]
            offs = {k: base + ch[k] for k in ('oE', 'oW', 'oQ', 'oC')}
            for jj in range(ch['nu']):
                u = su[ch['u0']+jj]
                r = u['i']*NCORES + cc
                cb = u['cb']
                pe, pv, cc_ = u['pe'], u['pv'], u['c']
                for m in range(N):
                    el = elists[(r, cb)][m]
                    vl = vlists[(r, cb)][m]
                    cl = clists[r][m]
                    sl = m*S
                    c0 = offs['oE'] + m*pe
                    bake(Wd, c0, r, cb, 'E', sl+el, 6)
                    Wd[2, c0+len(el):c0+pe] = BF16(BIGD)
                    c0 = offs['oW'] + m*pv
                    bake(Wd, c0, r, cb, 'W', sl+vl, 6)
                    Wd[2, c0+len(vl):c0+pv] = BF16(BIGD)
                    c0 = offs['oQ'] + m*pe
                    bake(Wd, c0, r, cb, 'Q', sl+el, 3)
                    c0 = offs['oC'] + m*cc_
                    bake(Wd, c0, r, cb, 'C', sl+cl, 3)
                    ck2[cc, (ch['u0']+jj)*8 + m] = 2*ndn[r][m] - len(cl)
                offs['oE'] += 8*pe
                offs['oW'] += 8*pv
                offs['oQ'] += 8*pe
                offs['oC'] += 8*cc_

    al8 = (-a_s).astype(np.float32)                 # (8,)
    ca8 = (c_s.T * a_s[None, :]).astype(np.float32)  # (3, 8)

    return dict(su=su, order_u=order_u, chunks=chunks, choff=choff,
                Wcore=Wcore, X18=X18, ck2=ck2,
                al8=al8, ca8=ca8, zeroE=zeroE, zeroW=zeroW,
                ESLAB=ESLAB, WSLAB=WSLAB, GSLAB=GSLAB, SCR=SCR,
                TOTW=TOTW, ETOT=ETOT, a_s=a_s, c_s=c_s)


# ----------------------------------------------------------- numpy emulator
def _bf(x):
    return x.astype(BF16).astype(np.float32)


def _emulate(pre, core):
    X = pre['X18'].astype(np.float32)
    Wd = pre['Wcore'][core].astype(np.float32)
    mindE = np.full((128, NSL), BIGD, np.float32)
    mindW = np.full((128, NSL), BIGD, np.float32)
    sEa = np.zeros((128, pre['ETOT']), np.float32)
    sS = np.zeros((128, NSL), np.float32)
    for ci, ch in enumerate(pre['chunks']):
        base = pre['choff'][ci]
        psum = np.zeros((128, ch['tot']), np.float32)
        for (c0, c1, cbv) in ch['mm']:
            psum[:, c0:c1] = X[:, cbv, :].T @ Wd[:, base+c0:base+c1]
        rq = _bf(np.maximum(psum[:, ch['oQ']:ch['oQ']+ch['nQ']], 0.0))
        psum[:, ch['oE']:ch['oE']+ch['nE']] += rq
        sg = _bf(np.sign(psum[:, ch['oC']:ch['oC']+ch['nC']]))
        for (qty, off, G, w, dst, doff) in ch['lvl0']:
            A = psum[:, off:off+G*w].reshape(128, G, w)
            out = _bf(A.min(axis=2))
            (mindE if dst == 'mE' else mindW)[:, doff:doff+G] = out
        goff0 = ch['cfold'][0][0] if ch['cfold'] else 0
        for (goff, G, w, doff) in ch['cfold']:
            A = sg[:, goff-goff0:goff-goff0+G*w].reshape(128, G, w)
            sS[:, doff:doff+G] = A.sum(axis=2)
    mind2 = _bf(np.minimum(mindE, mindW))
    m0 = np.maximum(mind2, 0.0)
    sd = _bf(np.sqrt(m0 + EPS))
    eq = (sS == pre['ck2'][core][None, :]).astype(np.float32)
    sgn = eq*2.0 - 1.0
    sdf = _bf(sgn*sd)
    cov = 1.0/(1.0 + np.exp(np.clip(100.0*sdf, -80, 80)))
    cov = _bf(cov).reshape(128, NU, N)
    uu = _bf(_bf(cov*_bf(pre['al8'])[None, None, :]) + 1.0)
    uu[:, :, 0] = 0.0
    out = np.zeros((128, 3, NU), np.float32)
    for chn in range(3):
        dd = _bf(cov*_bf(pre['ca8'][chn])[None, None, :])
        st = np.zeros((128, NU), np.float32)
        for sl in range(N):
            st = _bf(uu[:, :, sl]*st + dd[:, :, sl])
        out[:, chn] = np.clip(st, 0.0, 1.0)
    return out


def _assemble(pre, outs):
    img = np.empty((HW, HW, 3), np.float32)
    for cc in range(NCORES):
        o = outs[cc]
        for j in range(NU):
            u = pre['su'][j]
            r = u['i']*NCORES + cc
            cb = u['cb']
            img[r, cb*128:(cb+1)*128, :] = o[:, :, j]
    return img


# ------------------------------------------------------------- bass program
def _build_program(pre):
    import concourse.bass as bass
    import concourse.bacc as bacc
    import concourse.mybir as mybir
    from concourse import tile

    dt = mybir.dt.float32
    bt = mybir.dt.bfloat16
    AF = mybir.ActivationFunctionType
    ALU = mybir.AluOpType
    AX = mybir.AxisListType

    chunks, choff = pre['chunks'], pre['choff']

    nc = bacc.Bacc()
    w_d = nc.declare_dram_parameter("w", [18, pre['TOTW']], bt, isOutput=False)
    xf_d = nc.declare_dram_parameter("xfeat", [18, CB, 128], bt,
                                     isOutput=False)
    ck2_d = nc.declare_dram_parameter("ck2", [128, NSL], dt, isOutput=False)
    al8_d = nc.declare_dram_parameter("al8", [128, N], bt, isOutput=False)
    ca8_d = nc.declare_dram_parameter("ca8", [128, 3, N], bt, isOutput=False)
    idf_d = nc.declare_dram_parameter("identf", [128, 128], dt,
                                      isOutput=False)
    cst_d = nc.declare_dram_parameter("consts", [128, 8], dt, isOutput=False)
    out_d = nc.declare_dram_parameter("out", [3, NU, 128], dt, isOutput=True)

    with tile.TileContext(nc) as tc:
        with (
            tc.tile_pool(name="const", bufs=1) as cpool,
            tc.tile_pool(name="wpool", bufs=4) as wpool,
            tc.tile_pool(name="rqp", bufs=4) as rqp,
            tc.tile_pool(name="slabs", bufs=1) as slabs,
            tc.tile_pool(name="work", bufs=2) as work,
            tc.tile_pool(name="psc", bufs=2, space=bass.MemorySpace.PSUM) as psc,
            tc.tile_pool(name="pst", bufs=1, space=bass.MemorySpace.PSUM) as pst,
        ):
            # critical-path consts first; fat consts are DMA'd mid-loop
            xfeat = cpool.tile([18, CB, 128], bt)
            nc.sync.dma_start(xfeat[:], xf_d[:])
            cst = cpool.tile([128, 8], dt)
            c_eps = cst[:, 0:1]
            identf = cpool.tile([128, 128], dt)
            ck2t = cpool.tile([128, NSL], dt)
            al8 = cpool.tile([128, N], bt)
            ca8 = cpool.tile([128, 3, N], bt)

            sG = slabs.tile([128, pre['GSLAB']], bt)
            sEa = slabs.tile([128, pre['ETOT']], bt)
            mE = slabs.tile([128, NSL], bt)
            mW = slabs.tile([128, NSL], bt)
            sS = slabs.tile([128, NSL], dt)
            for (off, ln) in pre['zeroE']:
                nc.vector.memset(mE[:, off:off+ln], BIGD)
            for (off, ln) in pre['zeroW']:
                nc.vector.memset(mW[:, off:off+ln], BIGD)

            smap = {'mE': mE, 'mW': mW}

            def view3(t, off, G, w):
                return t[:, off:off+G*w].rearrange("p (g w) -> p g w", w=w)

            # pre-sigmoid elementwise chain (no ACT tables) emitted per slot
            # range; the bulk runs mid-loop in Vector idle slack
            mind2 = slabs.tile([128, NSL], bt)
            m0 = slabs.tile([128, NSL], bt)
            eq = slabs.tile([128, NSL], bt)
            sgn = slabs.tile([128, NSL], bt)

            def emit_pre(s0, s1):
                nc.vector.tensor_tensor(mind2[:, s0:s1], mE[:, s0:s1],
                                        mW[:, s0:s1], ALU.min)
                nc.vector.tensor_scalar_max(m0[:, s0:s1], mind2[:, s0:s1],
                                            0.0)
                nc.vector.tensor_tensor(eq[:, s0:s1], sS[:, s0:s1],
                                        ck2t[:, s0:s1], ALU.is_equal)
                nc.vector.tensor_scalar(sgn[:, s0:s1], eq[:, s0:s1],
                                        2.0, -1.0, ALU.mult, ALU.add)

            PREK = min(16, len(chunks) - 2)
            chK = chunks[PREK]
            PRESL = (chK['u0'] + chK['nu'])*8

            # ---------------- main loop
            g_off = 0
            for ci, ch in enumerate(chunks):
                base = choff[ci]
                wt = wpool.tile([18, ch['tot']], bt, tag="w")
                nc.sync.dma_start(wt[:], w_d[:, base:base+ch['tot']])
                ps = psc.tile([128, ch['tot']], dt, tag="ps")
                for (c0, c1, cbv) in ch['mm']:
                    nc.tensor.matmul(ps[:, c0:c1], xfeat[:, cbv, :],
                                     wt[:, c0:c1], start=True, stop=True)
                if ch['nQ']:
                    # E and Q zones are adjacent: one relu covers both
                    # (relu on E only clips negative rounding noise)
                    ebrq = rqp.tile([128, ch['nE']+ch['nQ']], bt, tag="ebrq")
                    nc.scalar.activation(ebrq[:],
                                         ps[:, ch['oE']:ch['oE']+ch['nE'] +
                                            ch['nQ']],
                                         AF.Relu)
                if ch['nC']:
                    nc.scalar.sign(
                        sG[:, g_off:g_off+ch['nC']],
                        ps[:, ch['oC']:ch['oC']+ch['nC']])
                # V queue: psum-only consumers (W reduces, C folds) first so
                # they overlap the Scalar relu/copy chain; E path after
                for (qty, off, G, w, dst, doff) in ch['lvl0']:
                    if qty == 'W':
                        nc.vector.tensor_reduce(
                            smap[dst][:, doff:doff+G], view3(ps, off, G, w),
                            AX.X, ALU.min)
                if ch['nQ']:
                    nc.vector.tensor_tensor(
                        sEa[:, ch['e_base']:ch['e_base']+ch['nE']],
                        ebrq[:, 0:ch['nE']],
                        ebrq[:, ch['nE']:ch['nE']+ch['nQ']], ALU.add)
                for (qty, off, G, w, dst, doff) in ch['lvl0']:
                    if qty == 'E':
                        so = ch['e_base'] + (off - ch['oE'])
                        nc.vector.tensor_reduce(
                            smap[dst][:, doff:doff+G], view3(sEa, so, G, w),
                            AX.X, ALU.min)
                for (goff, G, w, doff) in ch['cfold']:
                    nc.vector.tensor_reduce(
                        sS[:, doff:doff+G], view3(sG, goff, G, w),
                        AX.X, ALU.add)
                g_off += ch['nC']
                if ci == 3:
                    nc.sync.dma_start(cst[:], cst_d[:])
                    nc.sync.dma_start(ck2t[:], ck2_d[:])
                    nc.sync.dma_start(al8[:], al8_d[:])
                    nc.sync.dma_start(ca8[:], ca8_d[:])
                    nc.sync.dma_start(identf[:], idf_d[:])
                if ci == PREK:
                    emit_pre(0, PRESL)

            # ---------------- end phase, pipelined in two slot batches:
            # batch 1's sqrt starts right after the last relu while batch 2's
            # pre-chain still runs; each stage of one batch overlaps the
            # other batch's neighbor stage
            emit_pre(PRESL, NSL)
            sd = slabs.tile([128, NSL], bt)
            sdf = slabs.tile([128, NSL], bt)
            cov = slabs.tile([128, NSL], bt)
            uu = slabs.tile([128, NU, N], bt)
            och = slabs.tile([128, 3, NU], dt)
            d1_0 = slabs.tile([128, NU, N], bt)
            d1_1 = slabs.tile([128, NU, N], bt)
            d1_2 = slabs.tile([128, NU, N], bt)
            sc_0 = slabs.tile([128, NU, N], bt)
            sc_1 = slabs.tile([128, NU, N], bt)
            sc_2 = slabs.tile([128, NU, N], bt)
            d1s = [d1_0, d1_1, d1_2]
            scs = [sc_0, sc_1, sc_2]
            cov3 = cov[:].rearrange("p (u n) -> p u n", n=N)
            al8b = al8[:].rearrange("p (u n) -> p u n", u=1)\
                .broadcast_to([128, NU, N])
            batches = [(0, PRESL), (PRESL, NSL)]
            for (s0, s1) in batches:
                nc.scalar.activation(sd[:, s0:s1], m0[:, s0:s1], AF.Sqrt,
                                     bias=c_eps)
            for (s0, s1) in batches:
                nc.vector.tensor_tensor(sdf[:, s0:s1], sgn[:, s0:s1],
                                        sd[:, s0:s1], ALU.mult)
            for (s0, s1) in batches:
                nc.scalar.activation(cov[:, s0:s1], sdf[:, s0:s1],
                                     AF.Sigmoid, scale=-100.0)
                u0, u1 = s0//N, s1//N
                nc.vector.tensor_tensor(uu[:, u0:u1], cov3[:, u0:u1],
                                        al8b[:, u0:u1], ALU.mult)
                nc.vector.tensor_scalar(uu[:, u0:u1], uu[:, u0:u1],
                                        1.0, None, ALU.add)
                nc.vector.memset(uu[:, u0:u1, 0:1], 0.0)
                for chn in range(3):
                    ca8b = ca8[:, chn, :]\
                        .rearrange("p (u n) -> p u n", u=1)\
                        .broadcast_to([128, NU, N])
                    nc.vector.tensor_tensor(d1s[chn][:, u0:u1],
                                            cov3[:, u0:u1],
                                            ca8b[:, u0:u1], ALU.mult)
            # scan -> clip -> transpose -> copy per channel so the output
            # path of channel k overlaps channel k+1's scan
            obig = work.tile([128, 3, 128], dt, tag="obig")
            osml = work.tile([16, 3, 128], dt, tag="osml")
            uflat = uu[:].rearrange("p u n -> p (u n)")
            for chn in range(3):
                nc.vector.tensor_tensor_scan(
                    scs[chn][:].rearrange("p u n -> p (u n)"), uflat,
                    d1s[chn][:].rearrange("p u n -> p (u n)"), 0.0,
                    ALU.mult, ALU.add)
                nc.vector.tensor_scalar(och[:, chn, :], scs[chn][:, :, N-1],
                                        0.0, 1.0, ALU.max, ALU.min)
                t1 = pst.tile([128, 128], dt, tag="t1")
                nc.tensor.transpose(t1[:], och[:, chn, 0:128], identf[:])
                nc.scalar.copy(obig[:, chn, :], t1[:])
                t2 = pst.tile([16, 128], dt, tag="t2")
                nc.tensor.transpose(t2[:], och[:, chn, 128:NU], identf[:])
                nc.scalar.copy(osml[:, chn, :], t2[:])
            nc.sync.dma_start(
                out_d[:, 0:128, :].rearrange("c u p -> u c p"), obig[:])
            nc.sync.dma_start(
                out_d[:, 128:NU, :].rearrange("c u p -> u c p"), osml[:])

    nc.compile()
    return nc


# ---------------------------------------------------------------- fallback
def _numpy_reference(P, c, alpha, alive, z, csg, width, height):
    P = np.asarray(P, np.float32)
    sig = 1.0 / (1.0 + np.exp(-np.asarray(alive, np.float64)))
    eff_alpha = np.where(sig > 0.1, np.asarray(alpha, np.float64), 0.0)
    order = np.argsort(np.asarray(z, np.float64), kind='stable')
    P_s, c_s = P[order], np.asarray(c, np.float64)[order]
    a_s, csg_s = eff_alpha[order], np.asarray(csg, bool)[order]
    poly = _bezier_to_polyline(P_s.astype(np.float64))
    a = poly
    b = np.roll(poly, -1, axis=1)
    y = np.linspace(0, 1, height)
    x = np.linspace(0, 1, width)
    gx, gy = np.meshgrid(x, y)
    p = np.stack([gx, gy], -1)[None, None]
    av = a[:, :, None, None, :]
    bv = b[:, :, None, None, :]
    ab = bv - av
    ap = p - av
    t = np.clip((ap*ab).sum(-1) / ((ab*ab).sum(-1) + EPS), 0, 1)
    diff = p - (av + t[..., None]*ab)
    dist = np.sqrt((diff*diff).sum(-1).min(1) + EPS)
    ay_, by_, py_ = av[..., 1], bv[..., 1], p[..., 1]
    ax_, bx_, px_ = av[..., 0], bv[..., 0], p[..., 0]
    up = (ay_ <= py_) & (py_ < by_)
    dn = (ay_ > py_) & (py_ >= by_)
    left = (bx_-ax_)*(py_-ay_) - (px_-ax_)*(by_-ay_) > 0
    w = np.where(up & left, 1.0, 0.0) + np.where(dn & ~left, -1.0, 0.0)
    wn = w.sum(1)
    sdf = np.where(wn != 0, -dist, dist)
    cov = 1.0/(1.0 + np.exp(np.clip(sdf/0.01, -80, 80)))
    la_all = cov * a_s[:, None, None]
    rgb = np.zeros((height, width, 3))
    ca = np.zeros((height, width, 1))
    for s in range(len(a_s)):
        la = la_all[s][..., None]
        if csg_s[s]:
            ca2 = ca*(1-la)
            rgb = rgb * (ca2 > 0)
            ca = ca2
        else:
            out_a = la + ca*(1-la)
            safe = np.where(out_a > 0, out_a, 1.0)
            rgb = np.where(out_a > 0, (c_s[s]*la + rgb*ca*(1-la))/safe, 0.0)
            ca = out_a
    return np.clip(rgb*ca, 0, 1).astype(np.float32)


# ------------------------------------------------------------------ driver
LAST_RESULT = None


def kernel(P, c, alpha, alive, z, csg, width, height):
    global LAST_RESULT
    width = int(width)
    height = int(height)
    if width != HW or height != HW or np.asarray(csg).any():
        return _numpy_reference(P, c, alpha, alive, z, csg, width, height)

    pre = _precompute(P, c, alpha, alive, z)
    nc = _build_program(pre)

    from concourse.bass_utils import run_bass_kernel_spmd

    identf = np.eye(128, dtype=np.float32)
    cvals = [EPS] + [0.0]*7
    consts = np.broadcast_to(np.asarray(cvals, np.float32)[None, :],
                             (128, 8)).copy()
    al8 = np.broadcast_to(pre['al8'][None, :].astype(BF16), (128, N)).copy()
    ca8 = np.ascontiguousarray(np.broadcast_to(
        pre['ca8'][None, :, :].astype(BF16), (128, 3, N)))
    in_maps = []
    for cc in range(NCORES):
        ck2 = np.broadcast_to(pre['ck2'][cc][None, :].astype(np.float32),
                              (128, NSL)).copy()
        in_maps.append(dict(w=np.ascontiguousarray(pre['Wcore'][cc]),
                            xfeat=pre['X18'], ck2=ck2, al8=al8, ca8=ca8,
                            identf=identf, consts=consts))

    trace = bool(int(os.environ.get('DIFFRAST_TRACE', '0')))
    res = run_bass_kernel_spmd(nc, in_maps, core_ids=list(range(NCORES)),
                               trace=trace)
    LAST_RESULT = res

    outs = []
    for cc in range(NCORES):
        o = res.results[cc]['out']          # (3, NU, 128)
        outs.append(np.ascontiguousarray(o.transpose(2, 0, 1)))
    return _assemble(pre, outs)


if __name__ == '__main__':
    d = np.load(os.path.join(os.path.dirname(__file__), '_ref_cache.npz'))
    pre = _precompute(d['P'], d['c'], d['alpha'], d['alive'], d['z'])
    outs = [_emulate(pre, cc) for cc in range(NCORES)]
    img = _assemble(pre, outs)
    exp = d['expected']
    rel = np.linalg.norm(img - exp)/np.linalg.norm(exp)
    print('emulator rel err:', rel)
    print('chunks:', len(pre['chunks']),
          'mm:', sum(len(c['mm']) for c in pre['chunks']),
          'lvl0:', sum(len(c['lvl0']) for c in pre['chunks']),
          'cfold:', sum(len(c['cfold']) for c in pre['chunks']))



# revision 73
# speedup vs baseline: 1.0498x; 1.0161x over previous
"""Trainium2 Bass kernel v2 for the soft Bezier rasterizer.

dist^2(px, seg) = min(Wa, Wb, E + relu(Q)),  Q = |ab|^2 (t^2 - t);
E, Q, W (vertex dist^2), C (winding cross) are quadratics in px evaluated
by PE matmuls (split-bf16 weights, fp32 PSUM).  relu(Q) on ACT -> rq bf16;
an accumulating identity matmul adds rq onto the E columns in PSUM.
Per-(unit,shape) minima via bf16 tensor_tensor min fold-trees over
class-quantized group widths; units sorted by class so every fold level is
one big op.  Winding: g = [C>0] (tensor_scalar is_gt) + bf16 add-trees;
inside <=> sum != #dn.  Composite: premultiplied over-blend as a linear
recurrence via tensor_tensor_scan on a 9-slot [R, s0..s7] layout.
"""
import sys
import os
import numpy as np

for _p in ('/opt/trn_rl_repo',):
    if _p not in sys.path and os.path.isdir(_p):
        sys.path.insert(0, _p)

import ml_dtypes

BF16 = ml_dtypes.bfloat16

N = 8
S = 30
HW = 384
EPS = 1e-8
BIGD = 1e6
DTH = 0.04
NCORES = 8
RPC = HW // NCORES
CB = 3
NU = RPC * CB               # 144 units
NSL = NU * N                # 1152 slots
CHUNKCAP = 1536


def _bezier_to_polyline(cp, n_samples=S):
    t_global = np.linspace(0.0, 4.0 - 4.0 / n_samples, n_samples)
    seg = np.clip(np.floor(t_global).astype(np.int64), 0, 3)
    t = t_global - seg
    ti = 1.0 - t
    basis = np.stack([ti**3, 3*ti**2*t, 3*ti*t**2, t**3], axis=-1)
    idx = np.stack([seg*3, seg*3+1, seg*3+2, (seg*3+3) % 12], axis=-1)
    gathered = cp[:, idx, :]
    return np.einsum('sk,mskd->msd', basis, gathered)


def _split3(x):
    xh = x.astype(BF16).astype(np.float64)
    xm = (x - xh).astype(BF16).astype(np.float64)
    xl = (x - xh - xm).astype(BF16).astype(np.float64)
    return xh, xm, xl


_XTERM = [0, 0, 1, 0, 1, 2]
_WTERM = [0, 1, 0, 2, 1, 0]


def _qz(v, classes):
    for c in classes:
        if v <= c:
            return c
    return classes[-1]


def _runs(vals):
    """[(start, end, val)] maximal runs of equal values."""
    out = []
    s = 0
    for j in range(1, len(vals)+1):
        if j == len(vals) or vals[j] != vals[s]:
            out.append((s, j, vals[s]))
            s = j
    return out


def _precompute(P, c, alpha, alive, z):
    P64 = np.asarray(P, np.float64)
    sig_alive = 1.0 / (1.0 + np.exp(-np.asarray(alive, np.float64)))
    eff_alpha = np.where(sig_alive > 0.1, np.asarray(alpha, np.float64), 0.0)
    order = np.argsort(np.asarray(z, np.float64), kind='stable')
    P_s = P64[order]
    c_s = np.asarray(c, np.float64)[order]
    a_s = eff_alpha[order]

    poly = _bezier_to_polyline(P_s).astype(np.float32).astype(np.float64)
    a = poly
    b = np.roll(poly, -1, axis=1)
    ax, ay = a[..., 0].ravel(), a[..., 1].ravel()
    bx, by = b[..., 0].ravel(), b[..., 1].ravel()
    abx, aby = bx - ax, by - ay
    ab2 = abx**2 + aby**2 + EPS
    ylo, yhi = np.minimum(ay, by), np.maximum(ay, by)
    xlo, xhi = np.minimum(ax, bx), np.maximum(ax, bx)

    y = np.linspace(0.0, 1.0, HW)
    x = np.linspace(0.0, 1.0, HW)
    px0 = np.array([x[cb*128:(cb+1)*128].mean() for cb in range(CB)])

    elists, vlists, clists = {}, {}, {}
    ndn = np.zeros((HW, N))
    for r in range(HW):
        py = y[r]
        ys = (py > ylo - DTH) & (py < yhi + DTH)
        yv = np.abs(ay - py) <= DTH
        up = (ay <= py) & (py < by)
        dn = (ay > py) & (py >= by)
        cr = up | dn
        clists[r] = [np.nonzero(cr.reshape(N, S)[m])[0] for m in range(N)]
        ndn[r] = dn.reshape(N, S).sum(1)
        for cb in range(CB):
            x0b, x1b = x[cb*128], x[cb*128+127]
            xs = (xhi > x0b - DTH) & (xlo < x1b + DTH)
            xv = (ax > x0b - DTH) & (ax < x1b + DTH)
            es = (ys & xs).reshape(N, S)
            vs = (yv & xv).reshape(N, S)
            elists[(r, cb)] = [np.nonzero(es[m])[0] for m in range(N)]
            vlists[(r, cb)] = [np.nonzero(vs[m])[0] for m in range(N)]

    PE_CLS = (0, 2, 4, 8, 16)
    PV_CLS = (0, 2, 4, 8, 16)
    C_CLS = (2, 4, 8)
    units = []
    for i in range(RPC):
        rows = [i*NCORES + cc for cc in range(NCORES)]
        pc = max(len(clists[r][m]) for r in rows for m in range(N))
        cC = _qz(max(pc, 1), C_CLS)
        for cb in range(CB):
            pe = max(len(elists[(r, cb)][m]) for r in rows for m in range(N))
            pv = max(len(vlists[(r, cb)][m]) for r in rows for m in range(N))
            units.append(dict(i=i, cb=cb, pe=_qz(pe, PE_CLS),
                              pv=_qz(pv, PV_CLS), c=cC))
    order_u = sorted(range(NU), key=lambda j: (units[j]['pe'], units[j]['pv'],
                                               units[j]['c'], units[j]['cb'],
                                               units[j]['i']))
    su = [units[j] for j in order_u]

    # per-unit slab offsets (globally contiguous in sorted order)
    e_sl, w_sl, g_sl = [], [], []
    eo = wo = go = 0
    for u in su:
        e_sl.append(eo)
        w_sl.append(wo)
        g_sl.append(go)
        if u['pe'] > 2:
            eo += 8*(u['pe']//2)
        if u['pv'] > 2:
            wo += 8*(u['pv']//2)
        go += 8*u['c']
    ESLAB, WSLAB, GSLAB = max(eo, 1), max(wo, 1), max(go, 1)

    # chunks: consecutive sorted units, total psum cols <= CHUNKCAP
    chunks = []
    cur = None
    for j, u in enumerate(su):
        cols = 8*(2*u['pe'] + u['pv'] + u['c'])
        if cur is None or cur['cols'] + cols > CHUNKCAP:
            cur = dict(cols=0, u0=j, nu=0)
            chunks.append(cur)
        cur['cols'] += cols
        cur['nu'] += 1

    for ch in chunks:
        uu = su[ch['u0']:ch['u0']+ch['nu']]
        ch['nE'] = sum(8*u['pe'] for u in uu)
        ch['nW'] = sum(8*u['pv'] for u in uu)
        ch['nQ'] = ch['nE']
        ch['nC'] = sum(8*u['c'] for u in uu)
        ch['oW'], ch['oE'] = 0, ch['nW']
        ch['oQ'] = ch['oE'] + ch['nE']
        ch['oC'] = ch['oQ'] + ch['nQ']
        ch['tot'] = ch['oC'] + ch['nC']
        # mm pieces in data-arrival order: W (Vector's first consumer),
        # then E|Q (relu), then C (sign); split at cb changes and 512 grid
        pieces = []
        for zkey, cnt in (('oW', lambda u: 8*u['pv']),
                          ('oE', lambda u: 8*u['pe']),
                          ('oQ', lambda u: 8*u['pe']),
                          ('oC', lambda u: 8*u['c'])):
            zo = ch[zkey]
            off = zo
            c0 = zo
            for jj, u in enumerate(uu):
                if jj > 0 and uu[jj-1]['cb'] != u['cb']:
                    if off > c0:
                        pieces.append((c0, off, uu[jj-1]['cb']))
                    c0 = off
                off += cnt(u)
            if off > c0:
                pieces.append((c0, off, uu[-1]['cb']))
        split = []
        for (c0, c1, cbv) in pieces:
            p = c0
            while p < c1:
                nx = min(c1, (p//512+1)*512)
                split.append((p, nx, cbv))
                p = nx
        ch['mm'] = split
        acc = []
        p = ch['oE']
        while p < ch['oE']+ch['nE']:
            nx = min(ch['oE']+ch['nE'], (p//512+1)*512)
            acc.append((p, nx))
            p = nx
        ch['mmacc'] = acc
        # lvl0 fold pieces per class-run within chunk: (qty, psum off, G, w,
        # dstslab, dstoff).  dst 'E'/'W' slabs or 'mE'/'mW' when w==2.
        lvl0 = []
        for qty, zkey, key, mind in (('E', 'oE', 'pe', 'mE'),
                                     ('W', 'oW', 'pv', 'mW')):
            off = ch[zkey]
            vals = [u[key] for u in uu]
            for (s0, s1, v) in _runs(vals):
                G = (s1 - s0)*8
                if v > 0:
                    lvl0.append((qty, off, G, v, mind, (ch['u0']+s0)*8))
                off += G*v
        ch['lvl0'] = lvl0

    # C winding folds: per-chunk add-reduces over class runs (into sS fp32)
    g_off2 = 0
    for ch in chunks:
        uu = su[ch['u0']:ch['u0']+ch['nu']]
        cfold = []
        off = g_off2
        vals = [u['c'] for u in uu]
        for (s0, s1, v) in _runs(vals):
            G = (s1 - s0)*8
            cfold.append((off, G, v, (ch['u0']+s0)*8))
            off += G*v
        ch['cfold'] = cfold
        g_off2 = off

    # memset ranges for slots never written by folds (class 0)
    def zero_ranges(key):
        rngs = []
        for (s0, s1, v) in _runs([u[key] for u in su]):
            if v == 0:
                rngs.append((s0*8, (s1 - s0)*8))
        return rngs
    zeroE = zero_ranges('pe')
    zeroW = zero_ranges('pv')
    SCR = 1

    # ---- weights
    xf = np.stack([(x - np.repeat(px0, 128))**2,
                   x - np.repeat(px0, 128),
                   np.ones(HW)], 0)
    Xh, Xm, Xl = _split3(xf)
    X18 = np.zeros((18, CB, 128), BF16)
    for cb in range(CB):
        for t6 in range(6):
            X18[t6*3:(t6+1)*3, cb] = \
                (Xh, Xm, Xl)[_XTERM[t6]][:, cb*128:(cb+1)*128].astype(BF16)

    e_lin = aby*y[:, None] - abx*ax - aby*ay
    inv = 1.0 / ab2

    def col_coeffs(r, kind, g):
        py = y[r]
        n = len(g)
        Cq = np.zeros((3, n))
        if kind == 'E':
            e = e_lin[r][g]
            Cq[0] = 1.0 - abx[g]**2*inv[g]
            Cq[1] = -2*ax[g] - 2*abx[g]*e*inv[g]
            Cq[2] = ax[g]**2 + (py - ay[g])**2 - e**2*inv[g]
        elif kind == 'Q':
            e = e_lin[r][g]
            Cq[0] = abx[g]**2*inv[g]
            Cq[1] = 2*abx[g]*e*inv[g] - abx[g]
            Cq[2] = e**2*inv[g] - e
        elif kind == 'W':
            Cq[0] = 1.0
            Cq[1] = -2*ax[g]
            Cq[2] = ax[g]**2 + (py - ay[g])**2
        elif kind == 'C':
            Cq[1] = -aby[g]
            Cq[2] = abx[g]*(py - ay[g]) + ax[g]*aby[g]
        return Cq

    choff = []
    o = 0
    eb = 0
    for ch in chunks:
        choff.append(o)
        o += ch['tot']
        ch['e_base'] = eb
        eb += ch['nE']
    TOTW = o
    ETOT = max(eb, 1)
    Wcore = np.zeros((NCORES, 18, TOTW), BF16)
    ck2 = np.zeros((NCORES, NSL), np.float32)

    def bake(Wd, col0, r, cb, kind, g, nsplit):
        n = len(g)
        if n == 0:
            return
        Cq = col_coeffs(r, kind, g)
        p0 = px0[cb]
        A, B_, C0 = Cq
        Wq = np.stack([A, 2*A*p0 + B_, A*p0*p0 + B_*p0 + C0], 0)
        parts = _split3(Wq)
        for t6 in range(nsplit):
            Wd[t6*3:(t6+1)*3, col0:col0+n] = parts[_WTERM[t6]].astype(BF16)

    for cc in range(NCORES):
        Wd = Wcore[cc]
        for ci, ch in enumerate(chunks):
            base = choff[ci]
            offs = {k: base + ch[k] for k in ('oE', 'oW', 'oQ', 'oC')}
            for jj in range(ch['nu']):
                u = su[ch['u0']+jj]
                r = u['i']*NCORES + cc
                cb = u['cb']
                pe, pv, cc_ = u['pe'], u['pv'], u['c']
                for m in range(N):
                    el = elists[(r, cb)][m]
                    vl = vlists[(r, cb)][m]
                    cl = clists[r][m]
                    sl = m*S
                    c0 = offs['oE'] + m*pe
                    bake(Wd, c0, r, cb, 'E', sl+el, 6)
                    Wd[2, c0+len(el):c0+pe] = BF16(BIGD)
                    c0 = offs['oW'] + m*pv
                    bake(Wd, c0, r, cb, 'W', sl+vl, 6)
                    Wd[2, c0+len(vl):c0+pv] = BF16(BIGD)
                    c0 = offs['oQ'] + m*pe
                    bake(Wd, c0, r, cb, 'Q', sl+el, 3)
                    c0 = offs['oC'] + m*cc_
                    bake(Wd, c0, r, cb, 'C', sl+cl, 3)
                    ck2[cc, (ch['u0']+jj)*8 + m] = 2*ndn[r][m] - len(cl)
                offs['oE'] += 8*pe
                offs['oW'] += 8*pv
                offs['oQ'] += 8*pe
                offs['oC'] += 8*cc_

    al8 = (-a_s).astype(np.float32)                 # (8,)
    ca8 = (c_s.T * a_s[None, :]).astype(np.float32)  # (3, 8)

    return dict(su=su, order_u=order_u, chunks=chunks, choff=choff,
                Wcore=Wcore, X18=X18, ck2=ck2,
                al8=al8, ca8=ca8, zeroE=zeroE, zeroW=zeroW,
                ESLAB=ESLAB, WSLAB=WSLAB, GSLAB=GSLAB, SCR=SCR,
                TOTW=TOTW, ETOT=ETOT, a_s=a_s, c_s=c_s)


# ----------------------------------------------------------- numpy emulator
def _bf(x):
    return x.astype(BF16).astype(np.float32)


def _emulate(pre, core):
    X = pre['X18'].astype(np.float32)
    Wd = pre['Wcore'][core].astype(np.float32)
    mindE = np.full((128, NSL), BIGD, np.float32)
    mindW = np.full((128, NSL), BIGD, np.float32)
    sEa = np.zeros((128, pre['ETOT']), np.float32)
    sS = np.zeros((128, NSL), np.float32)
    for ci, ch in enumerate(pre['chunks']):
        base = pre['choff'][ci]
        psum = np.zeros((128, ch['tot']), np.float32)
        for (c0, c1, cbv) in ch['mm']:
            psum[:, c0:c1] = X[:, cbv, :].T @ Wd[:, base+c0:base+c1]
        rq = _bf(np.maximum(psum[:, ch['oQ']:ch['oQ']+ch['nQ']], 0.0))
        psum[:, ch['oE']:ch['oE']+ch['nE']] += rq
        sg = _bf(np.sign(psum[:, ch['oC']:ch['oC']+ch['nC']]))
        for (qty, off, G, w, dst, doff) in ch['lvl0']:
            A = psum[:, off:off+G*w].reshape(128, G, w)
            out = _bf(A.min(axis=2))
            (mindE if dst == 'mE' else mindW)[:, doff:doff+G] = out
        goff0 = ch['cfold'][0][0] if ch['cfold'] else 0
        for (goff, G, w, doff) in ch['cfold']:
            A = sg[:, goff-goff0:goff-goff0+G*w].reshape(128, G, w)
            sS[:, doff:doff+G] = A.sum(axis=2)
    mind2 = _bf(np.minimum(mindE, mindW))
    m0 = np.maximum(mind2, 0.0)
    sd = _bf(np.sqrt(m0 + EPS))
    eq = (sS == pre['ck2'][core][None, :]).astype(np.float32)
    sgn = eq*2.0 - 1.0
    sdf = _bf(sgn*sd)
    cov = 1.0/(1.0 + np.exp(np.clip(100.0*sdf, -80, 80)))
    cov = _bf(cov).reshape(128, NU, N)
    uu = _bf(_bf(cov*_bf(pre['al8'])[None, None, :]) + 1.0)
    uu[:, :, 0] = 0.0
    out = np.zeros((128, 3, NU), np.float32)
    for chn in range(3):
        dd = _bf(cov*_bf(pre['ca8'][chn])[None, None, :])
        st = np.zeros((128, NU), np.float32)
        for sl in range(N):
            st = _bf(uu[:, :, sl]*st + dd[:, :, sl])
        out[:, chn] = np.clip(st, 0.0, 1.0)
    return out


def _assemble(pre, outs):
    img = np.empty((HW, HW, 3), np.float32)
    for cc in range(NCORES):
        o = outs[cc]
        for j in range(NU):
            u = pre['su'][j]
            r = u['i']*NCORES + cc
            cb = u['cb']
            img[r, cb*128:(cb+1)*128, :] = o[:, :, j]
    return img


# ------------------------------------------------------------- bass program
def _build_program(pre):
    import concourse.bass as bass
    import concourse.bacc as bacc
    import concourse.mybir as mybir
    from concourse import tile

    dt = mybir.dt.float32
    bt = mybir.dt.bfloat16
    AF = mybir.ActivationFunctionType
    ALU = mybir.AluOpType
    AX = mybir.AxisListType

    chunks, choff = pre['chunks'], pre['choff']

    nc = bacc.Bacc()
    w_d = nc.declare_dram_parameter("w", [18, pre['TOTW']], bt, isOutput=False)
    xf_d = nc.declare_dram_parameter("xfeat", [18, CB, 128], bt,
                                     isOutput=False)
    ck2_d = nc.declare_dram_parameter("ck2", [128, NSL], dt, isOutput=False)
    al8_d = nc.declare_dram_parameter("al8", [128, N], bt, isOutput=False)
    ca8_d = nc.declare_dram_parameter("ca8", [128, 3, N], bt, isOutput=False)
    idf_d = nc.declare_dram_parameter("identf", [128, 128], dt,
                                      isOutput=False)
    cst_d = nc.declare_dram_parameter("consts", [128, 8], dt, isOutput=False)
    out_d = nc.declare_dram_parameter("out", [3, NU, 128], dt, isOutput=True)

    with tile.TileContext(nc) as tc:
        with (
            tc.tile_pool(name="const", bufs=1) as cpool,
            tc.tile_pool(name="wpool", bufs=4) as wpool,
            tc.tile_pool(name="rqp", bufs=4) as rqp,
            tc.tile_pool(name="slabs", bufs=1) as slabs,
            tc.tile_pool(name="work", bufs=2) as work,
            tc.tile_pool(name="psc", bufs=2, space=bass.MemorySpace.PSUM) as psc,
            tc.tile_pool(name="pst", bufs=1, space=bass.MemorySpace.PSUM) as pst,
        ):
            # critical-path consts first; fat consts are DMA'd mid-loop
            xfeat = cpool.tile([18, CB, 128], bt)
            nc.sync.dma_start(xfeat[:], xf_d[:])
            cst = cpool.tile([128, 8], dt)
            c_eps = cst[:, 0:1]
            identf = cpool.tile([128, 128], dt)
            ck2t = cpool.tile([128, NSL], dt)
            al8 = cpool.tile([128, N], bt)
            ca8 = cpool.tile([128, 3, N], bt)

            sG = slabs.tile([128, pre['GSLAB']], bt)
            sEa = slabs.tile([128, pre['ETOT']], bt)
            mE = slabs.tile([128, NSL], bt)
            mW = slabs.tile([128, NSL], bt)
            sS = slabs.tile([128, NSL], dt)
            for (off, ln) in pre['zeroE']:
                nc.vector.memset(mE[:, off:off+ln], BIGD)
            for (off, ln) in pre['zeroW']:
                nc.vector.memset(mW[:, off:off+ln], BIGD)

            smap = {'mE': mE, 'mW': mW}

            def view3(t, off, G, w):
                return t[:, off:off+G*w].rearrange("p (g w) -> p g w", w=w)

            # pre-sigmoid elementwise chain (no ACT tables) emitted per slot
            # range; the bulk runs mid-loop in Vector idle slack
            mind2 = slabs.tile([128, NSL], bt)
            m0 = slabs.tile([128, NSL], bt)
            eq = slabs.tile([128, NSL], bt)
            sgn = slabs.tile([128, NSL], bt)

            def emit_pre(s0, s1):
                nc.vector.tensor_tensor(mind2[:, s0:s1], mE[:, s0:s1],
                                        mW[:, s0:s1], ALU.min)
                nc.vector.tensor_scalar_max(m0[:, s0:s1], mind2[:, s0:s1],
                                            0.0)
                nc.vector.tensor_tensor(eq[:, s0:s1], sS[:, s0:s1],
                                        ck2t[:, s0:s1], ALU.is_equal)
                nc.vector.tensor_scalar(sgn[:, s0:s1], eq[:, s0:s1],
                                        2.0, -1.0, ALU.mult, ALU.add)

            PREK = min(16, len(chunks) - 2)
            chK = chunks[PREK]
            PRESL = (chK['u0'] + chK['nu'])*8

            # ---------------- main loop
            g_off = 0
            for ci, ch in enumerate(chunks):
                base = choff[ci]
                wt = wpool.tile([18, ch['tot']], bt, tag="w")
                nc.sync.dma_start(wt[:], w_d[:, base:base+ch['tot']])
                ps = psc.tile([128, ch['tot']], dt, tag="ps")
                for (c0, c1, cbv) in ch['mm']:
                    nc.tensor.matmul(ps[:, c0:c1], xfeat[:, cbv, :],
                                     wt[:, c0:c1], start=True, stop=True)
                if ch['nQ']:
                    # E and Q zones are adjacent: one relu covers both
                    # (relu on E only clips negative rounding noise)
                    ebrq = rqp.tile([128, ch['nE']+ch['nQ']], bt, tag="ebrq")
                    nc.scalar.activation(ebrq[:],
                                         ps[:, ch['oE']:ch['oE']+ch['nE'] +
                                            ch['nQ']],
                                         AF.Relu)
                if ch['nC']:
                    nc.scalar.sign(
                        sG[:, g_off:g_off+ch['nC']],
                        ps[:, ch['oC']:ch['oC']+ch['nC']])
                # V queue: psum-only consumers (W reduces, C folds) first so
                # they overlap the Scalar relu/copy chain; E path after
                for (qty, off, G, w, dst, doff) in ch['lvl0']:
                    if qty == 'W':
                        nc.vector.tensor_reduce(
                            smap[dst][:, doff:doff+G], view3(ps, off, G, w),
                            AX.X, ALU.min)
                if ch['nQ']:
                    nc.vector.tensor_tensor(
                        sEa[:, ch['e_base']:ch['e_base']+ch['nE']],
                        ebrq[:, 0:ch['nE']],
                        ebrq[:, ch['nE']:ch['nE']+ch['nQ']], ALU.add)
                for (qty, off, G, w, dst, doff) in ch['lvl0']:
                    if qty == 'E':
                        so = ch['e_base'] + (off - ch['oE'])
                        nc.vector.tensor_reduce(
                            smap[dst][:, doff:doff+G], view3(sEa, so, G, w),
                            AX.X, ALU.min)
                for (goff, G, w, doff) in ch['cfold']:
                    nc.vector.tensor_reduce(
                        sS[:, doff:doff+G], view3(sG, goff, G, w),
                        AX.X, ALU.add)
                g_off += ch['nC']
                if ci == 3:
                    nc.sync.dma_start(cst[:], cst_d[:])
                    nc.sync.dma_start(ck2t[:], ck2_d[:])
                    nc.sync.dma_start(al8[:], al8_d[:])
                    nc.sync.dma_start(ca8[:], ca8_d[:])
                    nc.sync.dma_start(identf[:], idf_d[:])
                if ci == PREK:
                    emit_pre(0, PRESL)

            # ---------------- end phase, pipelined in two slot batches:
            # batch 1's sqrt starts right after the last relu while batch 2's
            # pre-chain still runs; each stage of one batch overlaps the
            # other batch's neighbor stage
            emit_pre(PRESL, NSL)
            sd = slabs.tile([128, NSL], bt)
            sdf = slabs.tile([128, NSL], bt)
            cov = slabs.tile([128, NSL], bt)
            uu = slabs.tile([128, NU, N], bt)
            och = slabs.tile([128, 3, NU], dt)
            d1_0 = slabs.tile([128, NU, N], bt)
            d1_1 = slabs.tile([128, NU, N], bt)
            d1_2 = slabs.tile([128, NU, N], bt)
            sc_0 = slabs.tile([128, NU, N], bt)
            sc_1 = slabs.tile([128, NU, N], bt)
            sc_2 = slabs.tile([128, NU, N], bt)
            d1s = [d1_0, d1_1, d1_2]
            scs = [sc_0, sc_1, sc_2]
            cov3 = cov[:].rearrange("p (u n) -> p u n", n=N)
            al8b = al8[:].rearrange("p (u n) -> p u n", u=1)\
                .broadcast_to([128, NU, N])
            batches = [(0, PRESL), (PRESL, NSL)]
            for (s0, s1) in batches:
                nc.scalar.activation(sd[:, s0:s1], m0[:, s0:s1], AF.Sqrt,
                                     bias=c_eps)
            for (s0, s1) in batches:
                nc.vector.tensor_tensor(sdf[:, s0:s1], sgn[:, s0:s1],
                                        sd[:, s0:s1], ALU.mult)
            for (s0, s1) in batches:
                nc.scalar.activation(cov[:, s0:s1], sdf[:, s0:s1],
                                     AF.Sigmoid, scale=-100.0)
                u0, u1 = s0//N, s1//N
                nc.vector.tensor_tensor(uu[:, u0:u1], cov3[:, u0:u1],
                                        al8b[:, u0:u1], ALU.mult)
                nc.vector.tensor_scalar(uu[:, u0:u1], uu[:, u0:u1],
                                        1.0, None, ALU.add)
                nc.vector.memset(uu[:, u0:u1, 0:1], 0.0)
                for chn in range(3):
                    ca8b = ca8[:, chn, :]\
                        .rearrange("p (u n) -> p u n", u=1)\
                        .broadcast_to([128, NU, N])
                    nc.vector.tensor_tensor(d1s[chn][:, u0:u1],
                                            cov3[:, u0:u1],
                                            ca8b[:, u0:u1], ALU.mult)
            # scan -> clip -> transpose -> copy per channel so the output
            # path of channel k overlaps channel k+1's scan
            obig = work.tile([128, 3, 128], dt, tag="obig")
            osml = work.tile([16, 3, 128], dt, tag="osml")
            uflat = uu[:].rearrange("p u n -> p (u n)")
            for chn in range(3):
                nc.vector.tensor_tensor_scan(
                    scs[chn][:].rearrange("p u n -> p (u n)"), uflat,
                    d1s[chn][:].rearrange("p u n -> p (u n)"), 0.0,
                    ALU.mult, ALU.add)
                nc.vector.tensor_scalar(och[:, chn, :], scs[chn][:, :, N-1],
                                        0.0, 1.0, ALU.max, ALU.min)
                t1 = pst.tile([128, 128], dt, tag="t1")
                nc.tensor.transpose(t1[:], och[:, chn, 0:128], identf[:])
                nc.scalar.copy(obig[:, chn, :], t1[:])
                t2 = pst.tile([16, 128], dt, tag="t2")
                nc.tensor.transpose(t2[:], och[:, chn, 128:NU], identf[:])
                nc.scalar.copy(osml[:, chn, :], t2[:])
                # per-channel DMA: channel k's writeback overlaps channel
                # k+1's scan
                nc.sync.dma_start(out_d[chn, 0:128, :], obig[:, chn, :])
                nc.sync.dma_start(out_d[chn, 128:NU, :], osml[:, chn, :])

    nc.compile()
    return nc


# ---------------------------------------------------------------- fallback
def _numpy_reference(P, c, alpha, alive, z, csg, width, height):
    P = np.asarray(P, np.float32)
    sig = 1.0 / (1.0 + np.exp(-np.asarray(alive, np.float64)))
    eff_alpha = np.where(sig > 0.1, np.asarray(alpha, np.float64), 0.0)
    order = np.argsort(np.asarray(z, np.float64), kind='stable')
    P_s, c_s = P[order], np.asarray(c, np.float64)[order]
    a_s, csg_s = eff_alpha[order], np.asarray(csg, bool)[order]
    poly = _bezier_to_polyline(P_s.astype(np.float64))
    a = poly
    b = np.roll(poly, -1, axis=1)
    y = np.linspace(0, 1, height)
    x = np.linspace(0, 1, width)
    gx, gy = np.meshgrid(x, y)
    p = np.stack([gx, gy], -1)[None, None]
    av = a[:, :, None, None, :]
    bv = b[:, :, None, None, :]
    ab = bv - av
    ap = p - av
    t = np.clip((ap*ab).sum(-1) / ((ab*ab).sum(-1) + EPS), 0, 1)
    diff = p - (av + t[..., None]*ab)
    dist = np.sqrt((diff*diff).sum(-1).min(1) + EPS)
    ay_, by_, py_ = av[..., 1], bv[..., 1], p[..., 1]
    ax_, bx_, px_ = av[..., 0], bv[..., 0], p[..., 0]
    up = (ay_ <= py_) & (py_ < by_)
    dn = (ay_ > py_) & (py_ >= by_)
    left = (bx_-ax_)*(py_-ay_) - (px_-ax_)*(by_-ay_) > 0
    w = np.where(up & left, 1.0, 0.0) + np.where(dn & ~left, -1.0, 0.0)
    wn = w.sum(1)
    sdf = np.where(wn != 0, -dist, dist)
    cov = 1.0/(1.0 + np.exp(np.clip(sdf/0.01, -80, 80)))
    la_all = cov * a_s[:, None, None]
    rgb = np.zeros((height, width, 3))
    ca = np.zeros((height, width, 1))
    for s in range(len(a_s)):
        la = la_all[s][..., None]
        if csg_s[s]:
            ca2 = ca*(1-la)
            rgb = rgb * (ca2 > 0)
            ca = ca2
        else:
            out_a = la + ca*(1-la)
            safe = np.where(out_a > 0, out_a, 1.0)
            rgb = np.where(out_a > 0, (c_s[s]*la + rgb*ca*(1-la))/safe, 0.0)
            ca = out_a
    return np.clip(rgb*ca, 0, 1).astype(np.float32)


# ------------------------------------------------------------------ driver
LAST_RESULT = None


def kernel(P, c, alpha, alive, z, csg, width, height):
    global LAST_RESULT
    width = int(width)
    height = int(height)
    if width != HW or height != HW or np.asarray(csg).any():
        return _numpy_reference(P, c, alpha, alive, z, csg, width, height)

    pre = _precompute(P, c, alpha, alive, z)
    nc = _build_program(pre)

    from concourse.bass_utils import run_bass_kernel_spmd

    identf = np.eye(128, dtype=np.float32)
    cvals = [EPS] + [0.0]*7
    consts = np.broadcast_to(np.asarray(cvals, np.float32)[None, :],
                             (128, 8)).copy()
    al8 = np.broadcast_to(pre['al8'][None, :].astype(BF16), (128, N)).copy()
    ca8 = np.ascontiguousarray(np.broadcast_to(
        pre['ca8'][None, :, :].astype(BF16), (128, 3, N)))
    in_maps = []
    for cc in range(NCORES):
        ck2 = np.broadcast_to(pre['ck2'][cc][None, :].astype(np.float32),
                              (128, NSL)).copy()
        in_maps.append(dict(w=np.ascontiguousarray(pre['Wcore'][cc]),
                            xfeat=pre['X18'], ck2=ck2, al8=al8, ca8=ca8,
                            identf=identf, consts=consts))

    trace = bool(int(os.environ.get('DIFFRAST_TRACE', '0')))
    res = run_bass_kernel_spmd(nc, in_maps, core_ids=list(range(NCORES)),
                               trace=trace)
    LAST_RESULT = res

    outs = []
    for cc in range(NCORES):
        o = res.results[cc]['out']          # (3, NU, 128)
        outs.append(np.ascontiguousarray(o.transpose(2, 0, 1)))
    return _assemble(pre, outs)


if __name__ == '__main__':
    d = np.load(os.path.join(os.path.dirname(__file__), '_ref_cache.npz'))
    pre = _precompute(d['P'], d['c'], d['alpha'], d['alive'], d['z'])
    outs = [_emulate(pre, cc) for cc in range(NCORES)]
    img = _assemble(pre, outs)
    exp = d['expected']
    rel = np.linalg.norm(img - exp)/np.linalg.norm(exp)
    print('emulator rel err:', rel)
    print('chunks:', len(pre['chunks']),
          'mm:', sum(len(c['mm']) for c in pre['chunks']),
          'lvl0:', sum(len(c['lvl0']) for c in pre['chunks']),
          'cfold:', sum(len(c['cfold']) for c in pre['chunks']))

